# revision 16
# baseline (speedup 1.0000x reference)
"""Trainium2 Bass kernel for nn_DynamicImagePrimalDualNN.

T=128 primal-dual iterations over (2,1,160,160,32) with circular FD stencils.

Distribution: mb(2) x x-slabs(4) = 8 cores (ranks 0-3 = image 0, 4-7 = image
1; slab = rank%4). y and t stay core-local.

One AllGather per iteration: the dual variable qx is kept on the overlapping
slab [s-1, s+40) (one column redundantly computed by both neighbours), which
makes grad_GH fully local; only xbar needs halos, and both its planes
(first/last real column) are exchanged together in a single 4-rank AllGather
launched at the end of the previous iteration. Edge-column work is scheduled
late so the collective hides under bulk compute. All cross-iteration
dependencies are ordinary Tile-tracked tensor accesses - no manual sems.

Per-core layout: partitions p = (y%4)*32 + t (all 128 used);
free = (x_slot, yb). y/t stencils run on the TensorEngine via exact +-1
stationaries (circular yb handled by one pad column, circular t inside the
stationary); x stencils are DVE free-dim shifts.

Rescaled state so every scalar is an fp32 stt immediate:
  mt = p/sig,  Q = q/sig,  x0 raw.
  mt' = a*mt + a*xbar - cxn          (a = 1/(1+sig), cxn = a*xnoisy)
  Q'  = clip(Q + grad(xbar), lam/sig)
  x1  = x0 - c2*(mt' + div(Q'))      (c2 = ta*sig)
  xbar'= x1 + th*(x1 - x0)
"""

import math
from contextlib import ExitStack
from functools import lru_cache

import numpy as np

import concourse.bass as bass
import concourse.tile as tile
from concourse import bacc, mybir
from concourse.bass_utils import run_bass_kernel_spmd

F32 = mybir.dt.float32
AX = mybir.AluOpType

T_ITERS = 128
TRACE = False
_LAST_RESULTS = None
NXS = 40          # x-slab width per core
NYB = 40          # y blocks (y = 4*yb + my)
NCH = 10          # x-chunk width for PSUM-bank-sized matmuls
GROUPS = [[0, 1, 2, 3], [4, 5, 6, 7]]


def _pidx(m, t):
    return m * 32 + t


def _stationaries():
    """(128,128) matrices W[p_in, p_out]; matmul computes out[i] = sum_k W[k,i] in[k]."""
    I = np.eye(128, dtype=np.float32)
    dy = -np.eye(128, dtype=np.float32)
    cy = np.zeros((128, 128), np.float32)
    dt = -np.eye(128, dtype=np.float32)
    dyh = -np.eye(128, dtype=np.float32)
    cyh = np.zeros((128, 128), np.float32)
    dth = -np.eye(128, dtype=np.float32)
    for t in range(32):
        for m in range(3):
            dy[_pidx(m + 1, t), _pidx(m, t)] += 1.0
        cy[_pidx(0, t), _pidx(3, t)] = 1.0
        for m in range(1, 4):
            dyh[_pidx(m - 1, t), _pidx(m, t)] += 1.0
        cyh[_pidx(3, t), _pidx(0, t)] = 1.0
        for m in range(4):
            dt[_pidx(m, (t + 1) % 32), _pidx(m, t)] += 1.0
            dth[_pidx(m, (t - 1) % 32), _pidx(m, t)] += 1.0
    return dict(w_i=I, w_dy=dy, w_cy=cy, w_dt=dt, w_dyh=dyh, w_cyh=cyh,
                w_dth=dth)


def to_dev(v):
    """(xs, 160y, 32t) -> (128, xs, 40yb) with p=(y%4)*32+t."""
    xs = v.shape[0]
    return np.ascontiguousarray(
        v.reshape(xs, NYB, 4, 32).transpose(2, 3, 0, 1).reshape(128, xs, NYB))


def from_dev(v):
    """(128, xs, 40yb) -> (xs, 160y, 32t)."""
    xs = v.shape[1]
    return np.ascontiguousarray(
        v.reshape(4, 32, xs, NYB).transpose(2, 3, 0, 1).reshape(xs, 160, 32))


def _build_nc(scalars, T=T_ITERS):
    a_, c2, th = scalars
    nc = bacc.Bacc("TRN2", target_bir_lowering=False, debug=False,
                   num_devices=8)

    dp = {}
    for name in ("xb0", "x00", "mt0", "cxn"):
        dp[name] = nc.dram_tensor(name, [128, NXS, NYB], F32,
                                  kind="ExternalInput")
    # x-channel lambda covers the 41-wide overlap slab
    for name in ("lamx", "nlamx"):
        dp[name] = nc.dram_tensor(name, [128, NXS + 1, NYB], F32,
                                  kind="ExternalInput")
    for name in ("lamy", "nlamy", "lamt", "nlamt"):
        dp[name] = nc.dram_tensor(name, [128, NXS, NYB], F32,
                                  kind="ExternalInput")
    # (128, 8) one-hot masks over gathered slots (slot = rank_in_group*2 + e)
    for name in ("mskhi", "msklo"):
        dp[name] = nc.dram_tensor(name, [128, 8], F32, kind="ExternalInput")
    wnames = list(_stationaries().keys())
    for name in wnames:
        dp[name] = nc.dram_tensor(name, [128, 128], F32, kind="ExternalInput")
    out_dram = nc.dram_tensor("out", [128, NXS, NYB], F32,
                              kind="ExternalOutput")

    with tile.TileContext(nc) as tc, ExitStack() as es:
        state = es.enter_context(tc.tile_pool(name="state", bufs=1))
        xpool = es.enter_context(tc.tile_pool(name="xp", bufs=2))
        spool = es.enter_context(tc.tile_pool(name="scratch", bufs=2))
        dpool = es.enter_context(tc.tile_pool(name="dram", bufs=2,
                                              space="DRAM"))
        gpool = es.enter_context(tc.tile_pool(name="gath", bufs=2))
        psum = es.enter_context(
            tc.tile_pool(name="psum", bufs=8, space=bass.MemorySpace.PSUM))

        # xbar: x slots 0=halo_lo, 1..40 real, 41=halo_hi; yb col 40 = pad(yb0)
        xbar = state.tile([128, NXS + 2, NYB + 1], F32, tag="xbar")
        # qx on the 41-wide overlap slab (col j = global x s-1+j), no halos
        qx = state.tile([128, NXS + 1, NYB], F32, tag="qx")
        # qy: yb col 0 = pad(yb39), real yb at cols 1..40
        qy = state.tile([128, NXS, NYB + 1], F32, tag="qy")
        qt = state.tile([128, NXS, NYB], F32, tag="qt")
        mt = state.tile([128, NXS, NYB], F32, tag="mt")
        cxn = state.tile([128, NXS, NYB], F32, tag="cxn")
        lamx = state.tile([128, NXS + 1, NYB], F32, tag="lamx")
        nlamx = state.tile([128, NXS + 1, NYB], F32, tag="nlamx")
        lamy = state.tile([128, NXS, NYB], F32, tag="lamy")
        nlamy = state.tile([128, NXS, NYB], F32, tag="nlamy")
        lamt = state.tile([128, NXS, NYB], F32, tag="lamt")
        nlamt = state.tile([128, NXS, NYB], F32, tag="nlamt")
        mskhi = state.tile([128, 8], F32, tag="mskhi")
        msklo = state.tile([128, 8], F32, tag="msklo")
        W = {n: state.tile([128, 128], F32, tag=n, name=f"w_{n}")
             for n in wnames}

        nc.sync.dma_start(xbar[:, 1:41, 0:40], dp["xb0"][:])
        x0 = xpool.tile([128, NXS, NYB], F32, tag="x")
        nc.sync.dma_start(x0[:], dp["x00"][:])
        nc.sync.dma_start(mt[:], dp["mt0"][:])
        nc.sync.dma_start(cxn[:], dp["cxn"][:])
        for nm, tl in (("lamx", lamx), ("nlamx", nlamx), ("lamy", lamy),
                       ("nlamy", nlamy), ("lamt", lamt), ("nlamt", nlamt),
                       ("mskhi", mskhi), ("msklo", msklo)):
            nc.sync.dma_start(tl[:], dp[nm][:])
        for n in wnames:
            nc.sync.dma_start(W[n][:], dp[n][:])
        nc.vector.memset(qx[:], 0.0)
        nc.vector.memset(qy[:], 0.0)
        nc.vector.memset(qt[:], 0.0)
        nc.vector.tensor_copy(xbar[:, 1:41, 40:41], xbar[:, 1:41, 0:1])

        def exchange(round_idx):
            """AG of my (first,last) xbar planes; returns gathered sbuf tile."""
            bin_ = dpool.tile([2, 128, NYB], F32, tag="bin",
                              name=f"bin{round_idx}")
            bout = dpool.tile([8, 128, NYB], F32, tag="bout",
                              name=f"bout{round_idx}")
            nc.sync.dma_start(bin_[0], xbar[:, 1, 0:40])
            nc.sync.dma_start(bin_[1], xbar[:, 40, 0:40])
            nc.gpsimd.collective_compute(
                "AllGather", AX.bypass, replica_groups=GROUPS,
                ins=[bin_[:]], outs=[bout[:]])
            return bout

        def recv(bout):
            """DMA gathered planes to SBUF and mask-combine into xbar halos."""
            gath = gpool.tile([128, 8, NYB], F32, tag="gath")
            nc.sync.dma_start(gath[:], bout[:].transpose([1, 0, 2]))
            hi = spool.tile([128, NYB], F32, tag="hi")
            lo = spool.tile([128, NYB], F32, tag="lo")
            nc.vector.tensor_scalar(hi[:], gath[:, 0, :], mskhi[:, 0:1],
                                    None, AX.mult)
            nc.vector.tensor_scalar(lo[:], gath[:, 1, :], msklo[:, 1:2],
                                    None, AX.mult)
            for j in (1, 2, 3):
                nc.vector.scalar_tensor_tensor(
                    hi[:], gath[:, 2 * j, :], mskhi[:, 2 * j:2 * j + 1],
                    hi[:], AX.mult, AX.add)
                nc.vector.scalar_tensor_tensor(
                    lo[:], gath[:, 2 * j + 1, :],
                    msklo[:, 2 * j + 1:2 * j + 2], lo[:], AX.mult, AX.add)
            nc.vector.tensor_copy(xbar[:, 41, 0:40], hi[:])
            nc.vector.tensor_copy(xbar[:, 0, 0:40], lo[:])

        bout = exchange(0)

        for k in range(T):
            recv(bout)

            # --- qx chain: interior (cols 1..39) free of halos; edges last ---
            g = spool.tile([128, NXS + 1, NYB], F32, tag="g")
            nc.vector.tensor_sub(g[:, 1:40, :], xbar[:, 2:41, 0:40],
                                 xbar[:, 1:40, 0:40])
            nc.vector.tensor_add(g[:, 1:40, :], g[:, 1:40, :], qx[:, 1:40, :])
            nc.vector.tensor_tensor(qx[:, 1:40, :], g[:, 1:40, :],
                                    nlamx[:, 1:40, :], AX.max)
            nc.vector.tensor_tensor(qx[:, 1:40, :], qx[:, 1:40, :],
                                    lamx[:, 1:40, :], AX.min)
            for e, (xs0, xs1) in ((0, (1, 0)), (40, (41, 40))):
                sl = slice(e, e + 1)
                nc.vector.tensor_sub(g[:, sl, :], xbar[:, xs0:xs0 + 1, 0:40],
                                     xbar[:, xs1:xs1 + 1, 0:40])
                nc.vector.tensor_add(g[:, sl, :], g[:, sl, :], qx[:, sl, :])
                nc.vector.tensor_tensor(qx[:, sl, :], g[:, sl, :],
                                        nlamx[:, sl, :], AX.max)
                nc.vector.tensor_tensor(qx[:, sl, :], qx[:, sl, :],
                                        lamx[:, sl, :], AX.min)

            # --- qy/qt on PE ---
            ps_y, ps_t = [], []
            for c in range(4):
                sl = slice(1 + NCH * c, 1 + NCH * (c + 1))
                slq = slice(NCH * c, NCH * (c + 1))
                ps = psum.tile([128, NCH, NYB], F32, tag="ps")
                nc.tensor.matmul(ps[:], W["w_i"][:], qy[:, slq, 1:41],
                                 start=True, stop=False)
                nc.tensor.matmul(ps[:], W["w_dy"][:], xbar[:, sl, 0:40],
                                 start=False, stop=False)
                nc.tensor.matmul(ps[:], W["w_cy"][:], xbar[:, sl, 1:41],
                                 start=False, stop=True)
                ps_y.append(ps)
            for c in range(4):
                sl = slice(1 + NCH * c, 1 + NCH * (c + 1))
                slq = slice(NCH * c, NCH * (c + 1))
                ps = psum.tile([128, NCH, NYB], F32, tag="ps")
                nc.tensor.matmul(ps[:], W["w_i"][:], qt[:, slq, :],
                                 start=True, stop=False)
                nc.tensor.matmul(ps[:], W["w_dt"][:], xbar[:, sl, 0:40],
                                 start=False, stop=True)
                ps_t.append(ps)
            for c in range(4):
                slq = slice(NCH * c, NCH * (c + 1))
                sl1 = slice(1 + NCH * c, 1 + NCH * (c + 1))
                nc.vector.tensor_tensor(qy[:, slq, 1:41], ps_y[c][:],
                                        nlamy[:, slq, :], AX.max)
            nc.vector.tensor_tensor(qy[:, :, 1:41], qy[:, :, 1:41],
                                    lamy[:], AX.min)
            nc.vector.tensor_copy(qy[:, :, 0:1], qy[:, :, 40:41])
            for c in range(4):
                slq = slice(NCH * c, NCH * (c + 1))
                nc.vector.tensor_tensor(qt[:, slq, :], ps_t[c][:],
                                        nlamt[:, slq, :], AX.max)
            nc.vector.tensor_tensor(qt[:], qt[:], lamt[:], AX.min)

            # --- p-phase ---
            t1 = spool.tile([128, NXS, NYB], F32, tag="t1")
            nc.vector.scalar_tensor_tensor(t1[:], xbar[:, 1:41, 0:40], a_,
                                           cxn[:], AX.mult, AX.subtract)
            nc.vector.scalar_tensor_tensor(mt[:], mt[:], a_, t1[:],
                                           AX.mult, AX.add)

            # --- D + x-phase, edge chunks (0, 3) last only in xbar emit ---
            v = spool.tile([128, NXS, NYB], F32, tag="v")
            x1 = xpool.tile([128, NXS, NYB], F32, tag="x")
            for c in (1, 2, 0, 3):
                sl = slice(1 + NCH * c, 1 + NCH * (c + 1))
                slq = slice(NCH * c, NCH * (c + 1))          # qx[x-1]
                slq1 = slice(NCH * c + 1, NCH * (c + 1) + 1)  # qx[x]
                ps = psum.tile([128, NCH, NYB], F32, tag="ps")
                nc.tensor.matmul(ps[:], W["w_dyh"][:], qy[:, slq, 1:41],
                                 start=True, stop=False)
                nc.tensor.matmul(ps[:], W["w_cyh"][:], qy[:, slq, 0:40],
                                 start=False, stop=False)
                nc.tensor.matmul(ps[:], W["w_dth"][:], qt[:, slq, :],
                                 start=False, stop=True)
                nc.vector.tensor_add(ps[:], ps[:], qx[:, slq, :])
                nc.vector.tensor_sub(ps[:], ps[:], qx[:, slq1, :])
                nc.vector.tensor_add(v[:, slq, :], mt[:, slq, :], ps[:])
                nc.vector.scalar_tensor_tensor(
                    x1[:, slq, :], v[:, slq, :], -c2, x0[:, slq, :],
                    AX.mult, AX.add)

            if k < T - 1:
                vb = spool.tile([128, NXS, NYB], F32, tag="vb")
                # edge columns of xbar' first, then launch the exchange
                for col in (0, 39):
                    slc = slice(col, col + 1)
                    slx = slice(col + 1, col + 2)
                    nc.vector.tensor_sub(vb[:, slc, :], x1[:, slc, :],
                                         x0[:, slc, :])
                    nc.vector.scalar_tensor_tensor(
                        xbar[:, slx, 0:40], vb[:, slc, :], th, x1[:, slc, :],
                        AX.mult, AX.add)
                bout = exchange(k + 1)
                nc.vector.tensor_sub(vb[:, 1:39, :], x1[:, 1:39, :],
                                     x0[:, 1:39, :])
                nc.vector.scalar_tensor_tensor(
                    xbar[:, 2:40, 0:40], vb[:, 1:39, :], th, x1[:, 1:39, :],
                    AX.mult, AX.add)
                nc.vector.tensor_copy(xbar[:, 1:41, 40:41],
                                      xbar[:, 1:41, 0:1])
            x0 = x1

        nc.sync.dma_start(out_dram[:], x0[:])

    nc.compile()
    return nc


@lru_cache(maxsize=4)
def _compiled(scalars, T):
    return _build_nc(scalars, T)


def _make_in_maps(x, lambda_map, scalars, sig):
    stats = _stationaries()
    a_, c2, th = scalars
    in_maps = []
    for rank in range(8):
        mbi, pos = rank // 4, rank % 4
        s = pos * NXS
        xs = slice(s, s + NXS)
        xn = np.ascontiguousarray(x[mbi, 0, xs]).astype(np.float32)
        lam = lambda_map[mbi].astype(np.float32) / np.float32(sig)
        # x-channel lambda on the 41-wide overlap slab [s-1, s+40)
        idx = [(s - 1 + j) % 160 for j in range(NXS + 1)]
        lx = lam[0][idx]
        nxt, prv = (pos + 1) % 4, (pos - 1) % 4
        mhi = np.zeros((128, 8), np.float32)
        mlo = np.zeros((128, 8), np.float32)
        mhi[:, 2 * nxt] = 1.0        # next's first plane -> halo_hi
        mlo[:, 2 * prv + 1] = 1.0    # prev's last plane  -> halo_lo
        m = dict(
            xb0=to_dev(xn),
            x00=to_dev(xn),
            mt0=to_dev(xn / np.float32(sig)),
            cxn=to_dev(np.float32(a_) * xn),
            lamx=to_dev(lx), nlamx=to_dev(-lx),
            lamy=to_dev(lam[1][xs]), nlamy=to_dev(-lam[1][xs]),
            lamt=to_dev(lam[2][xs]), nlamt=to_dev(-lam[2][xs]),
            mskhi=mhi, msklo=mlo,
        )
        m.update({k: v.copy() for k, v in stats.items()})
        in_maps.append(m)
    return in_maps


def kernel(x, lambda_map, tau, sigma, theta):
    x = np.asarray(x, dtype=np.float32)
    lambda_map = np.asarray(lambda_map, dtype=np.float32)
    L = math.sqrt(13.0)
    sig = float(1.0 / (1.0 + math.exp(-float(np.asarray(sigma)[0])))) / L
    ta = float(1.0 / (1.0 + math.exp(-float(np.asarray(tau)[0])))) / L
    th = float(1.0 / (1.0 + math.exp(-float(np.asarray(theta)[0]))))
    a_ = 1.0 / (1.0 + sig)
    c2 = ta * sig
    scalars = tuple(float(np.float32(v)) for v in (a_, c2, th))

    nc = _compiled(scalars, T_ITERS)
    in_maps = _make_in_maps(x, lambda_map, scalars, sig)
    res = run_bass_kernel_spmd(nc, in_maps, core_ids=list(range(8)),
                               trace=TRACE)
    global _LAST_RESULTS
    _LAST_RESULTS = res

    out = np.zeros((2, 1, 160, 160, 32), np.float32)
    for rank in range(8):
        mbi, pos = rank // 4, rank % 4
        s = pos * NXS
        out[mbi, 0, s:s + NXS] = from_dev(res.results[rank]["out"])
    return out


# revision 18
# speedup vs baseline: 1.3778x; 1.3778x over previous
"""Trainium2 Bass kernel for nn_DynamicImagePrimalDualNN.

T=128 primal-dual iterations over (2,1,160,160,32) with circular FD stencils.

Distribution: mb(2) x x-slabs(4) = 8 cores (ranks 0-3 = image 0, 4-7 = image
1; slab = rank%4). y and t stay core-local.

One AllGather per iteration: the dual variable qx is kept on the overlapping
slab [s-1, s+40) (one column redundantly computed by both neighbours), which
makes grad_GH fully local; only xbar needs halos, and both its planes
(first/last real column) are exchanged together in a single 4-rank AllGather
launched at the end of the previous iteration. Edge-column work is scheduled
late so the collective hides under bulk compute. All cross-iteration
dependencies are ordinary Tile-tracked tensor accesses - no manual sems.

Per-core layout: partitions p = (y%4)*32 + t (all 128 used);
free = (x_slot, yb). y/t stencils run on the TensorEngine via exact +-1
stationaries (circular yb handled by one pad column, circular t inside the
stationary); x stencils are DVE free-dim shifts.

Rescaled state so every scalar is an fp32 stt immediate:
  mt = p/sig,  Q = q/sig,  x0 raw.
  mt' = a*mt + a*xbar - cxn          (a = 1/(1+sig), cxn = a*xnoisy)
  Q'  = clip(Q + grad(xbar), lam/sig)
  x1  = x0 - c2*(mt' + div(Q'))      (c2 = ta*sig)
  xbar'= x1 + th*(x1 - x0)
"""

import math
from contextlib import ExitStack
from functools import lru_cache

import numpy as np

import concourse.bass as bass
import concourse.tile as tile
from concourse import bacc, mybir
from concourse.bass_utils import run_bass_kernel_spmd

F32 = mybir.dt.float32
BF = mybir.dt.bfloat16
AX = mybir.AluOpType

T_ITERS = 128
TRACE = False
_LAST_RESULTS = None
NXS = 40          # x-slab width per core
NYB = 40          # y blocks (y = 4*yb + my)
NCH = 10          # x-chunk width for PSUM-bank-sized matmuls
GROUPS = [[0, 1, 2, 3], [4, 5, 6, 7]]


def _pidx(m, t):
    return m * 32 + t


def _stationaries():
    """(128,128) matrices W[p_in, p_out]; matmul computes out[i] = sum_k W[k,i] in[k]."""
    I = np.eye(128, dtype=np.float32)
    dy = -np.eye(128, dtype=np.float32)
    cy = np.zeros((128, 128), np.float32)
    dt = -np.eye(128, dtype=np.float32)
    dyh = -np.eye(128, dtype=np.float32)
    cyh = np.zeros((128, 128), np.float32)
    dth = -np.eye(128, dtype=np.float32)
    for t in range(32):
        for m in range(3):
            dy[_pidx(m + 1, t), _pidx(m, t)] += 1.0
        cy[_pidx(0, t), _pidx(3, t)] = 1.0
        for m in range(1, 4):
            dyh[_pidx(m - 1, t), _pidx(m, t)] += 1.0
        cyh[_pidx(3, t), _pidx(0, t)] = 1.0
        for m in range(4):
            dt[_pidx(m, (t + 1) % 32), _pidx(m, t)] += 1.0
            dth[_pidx(m, (t - 1) % 32), _pidx(m, t)] += 1.0
    return dict(w_i=I, w_ni=-I, w_dy=dy, w_cy=cy, w_dt=dt, w_dyh=dyh,
                w_cyh=cyh, w_dth=dth)


def to_dev(v):
    """(xs, 160y, 32t) -> (128, xs, 40yb) with p=(y%4)*32+t."""
    xs = v.shape[0]
    return np.ascontiguousarray(
        v.reshape(xs, NYB, 4, 32).transpose(2, 3, 0, 1).reshape(128, xs, NYB))


def from_dev(v):
    """(128, xs, 40yb) -> (xs, 160y, 32t)."""
    xs = v.shape[1]
    return np.ascontiguousarray(
        v.reshape(4, 32, xs, NYB).transpose(2, 3, 0, 1).reshape(xs, 160, 32))


def _build_nc(scalars, T=T_ITERS):
    a_, c2, th = scalars
    nc = bacc.Bacc("TRN2", target_bir_lowering=False, debug=False,
                   num_devices=8)

    dp = {}
    dp["xb0"] = nc.dram_tensor("xb0", [128, NXS, NYB], BF,
                               kind="ExternalInput")
    for name in ("x00", "mt0", "cxn"):
        dp[name] = nc.dram_tensor(name, [128, NXS, NYB], F32,
                                  kind="ExternalInput")
    # x-channel lambda covers the 41-wide overlap slab
    for name in ("lamx", "nlamx"):
        dp[name] = nc.dram_tensor(name, [128, NXS + 1, NYB], BF,
                                  kind="ExternalInput")
    for name in ("lamy", "nlamy", "lamt", "nlamt"):
        dp[name] = nc.dram_tensor(name, [128, NXS, NYB], BF,
                                  kind="ExternalInput")
    # (128, 8) one-hot masks over gathered slots (slot = rank_in_group*2 + e)
    for name in ("mskhi", "msklo"):
        dp[name] = nc.dram_tensor(name, [128, 8], F32, kind="ExternalInput")
    wnames = list(_stationaries().keys())
    for name in wnames:
        dp[name] = nc.dram_tensor(name, [128, 128], BF, kind="ExternalInput")
    out_dram = nc.dram_tensor("out", [128, NXS, NYB], F32,
                              kind="ExternalOutput")

    with tile.TileContext(nc) as tc, ExitStack() as es:
        state = es.enter_context(tc.tile_pool(name="state", bufs=1))
        xpool = es.enter_context(tc.tile_pool(name="xp", bufs=2))
        spool = es.enter_context(tc.tile_pool(name="scratch", bufs=2))
        dpool = es.enter_context(tc.tile_pool(name="dram", bufs=2,
                                              space="DRAM"))
        gpool = es.enter_context(tc.tile_pool(name="gath", bufs=2))
        psum = es.enter_context(
            tc.tile_pool(name="psum", bufs=8, space=bass.MemorySpace.PSUM))

        # xbar: x slots 0=halo_lo, 1..40 real, 41=halo_hi; yb col 40 =
        # pad(yb0), col 41 unused (even stride keeps bf16 2x alignment)
        xbar = state.tile([128, NXS + 2, NYB + 2], BF, tag="xbar")
        # qx on the 41-wide overlap slab (col j = global x s-1+j), no halos
        qx = state.tile([128, NXS + 1, NYB], BF, tag="qx")
        # qy: yb col 0 = pad(yb39), real yb at cols 1..40, col 41 unused
        qy = state.tile([128, NXS, NYB + 2], BF, tag="qy")
        qt = state.tile([128, NXS, NYB], BF, tag="qt")
        mt = state.tile([128, NXS, NYB], F32, tag="mt")
        cxn = state.tile([128, NXS, NYB], F32, tag="cxn")
        lamx = state.tile([128, NXS + 1, NYB], BF, tag="lamx")
        nlamx = state.tile([128, NXS + 1, NYB], BF, tag="nlamx")
        lamy = state.tile([128, NXS, NYB], BF, tag="lamy")
        nlamy = state.tile([128, NXS, NYB], BF, tag="nlamy")
        lamt = state.tile([128, NXS, NYB], BF, tag="lamt")
        nlamt = state.tile([128, NXS, NYB], BF, tag="nlamt")
        mskhi = state.tile([128, 8], F32, tag="mskhi")
        msklo = state.tile([128, 8], F32, tag="msklo")
        W = {n: state.tile([128, 128], BF, tag=n, name=f"w_{n}")
             for n in wnames}

        nc.sync.dma_start(xbar[:, 1:41, 0:40], dp["xb0"][:])
        x0 = xpool.tile([128, NXS, NYB], F32, tag="x")
        nc.sync.dma_start(x0[:], dp["x00"][:])
        nc.sync.dma_start(mt[:], dp["mt0"][:])
        nc.sync.dma_start(cxn[:], dp["cxn"][:])
        for nm, tl in (("lamx", lamx), ("nlamx", nlamx), ("lamy", lamy),
                       ("nlamy", nlamy), ("lamt", lamt), ("nlamt", nlamt),
                       ("mskhi", mskhi), ("msklo", msklo)):
            nc.sync.dma_start(tl[:], dp[nm][:])
        for n in wnames:
            nc.sync.dma_start(W[n][:], dp[n][:])
        nc.vector.memset(qx[:], 0.0)
        nc.vector.memset(qy[:], 0.0)
        nc.vector.memset(qt[:], 0.0)
        nc.vector.tensor_copy(xbar[:, 1:41, 40:41], xbar[:, 1:41, 0:1])

        def exchange(round_idx):
            """AG of my (first,last) xbar planes; returns gathered sbuf tile."""
            bin_ = dpool.tile([2, 128, NYB], BF, tag="bin",
                              name=f"bin{round_idx}")
            bout = dpool.tile([8, 128, NYB], BF, tag="bout",
                              name=f"bout{round_idx}")
            nc.sync.dma_start(bin_[0], xbar[:, 1, 0:40])
            nc.sync.dma_start(bin_[1], xbar[:, 40, 0:40])
            nc.gpsimd.collective_compute(
                "AllGather", AX.bypass, replica_groups=GROUPS,
                ins=[bin_[:]], outs=[bout[:]])
            return bout

        def recv(bout):
            """DMA gathered planes to SBUF and mask-combine into xbar halos."""
            gath = gpool.tile([128, 8, NYB], BF, tag="gath")
            nc.sync.dma_start(gath[:], bout[:].transpose([1, 0, 2]))
            hi = spool.tile([128, NYB], BF, tag="hi")
            lo = spool.tile([128, NYB], BF, tag="lo")
            nc.vector.tensor_scalar(hi[:], gath[:, 0, :], mskhi[:, 0:1],
                                    None, AX.mult)
            nc.vector.tensor_scalar(lo[:], gath[:, 1, :], msklo[:, 1:2],
                                    None, AX.mult)
            for j in (1, 2, 3):
                nc.vector.scalar_tensor_tensor(
                    hi[:], gath[:, 2 * j, :], mskhi[:, 2 * j:2 * j + 1],
                    hi[:], AX.mult, AX.add)
                nc.vector.scalar_tensor_tensor(
                    lo[:], gath[:, 2 * j + 1, :],
                    msklo[:, 2 * j + 1:2 * j + 2], lo[:], AX.mult, AX.add)
            nc.vector.tensor_copy(xbar[:, 41, 0:40], hi[:])
            nc.vector.tensor_copy(xbar[:, 0, 0:40], lo[:])

        bout = exchange(0)

        for k in range(T):
            recv(bout)

            # --- qx chain: interior (cols 1..39) free of halos; edges last ---
            g = spool.tile([128, NXS + 1, NYB], BF, tag="g")
            nc.vector.tensor_sub(g[:, 1:40, :], xbar[:, 2:41, 0:40],
                                 xbar[:, 1:40, 0:40])
            nc.vector.tensor_add(g[:, 1:40, :], g[:, 1:40, :], qx[:, 1:40, :])
            nc.vector.tensor_tensor(qx[:, 1:40, :], g[:, 1:40, :],
                                    nlamx[:, 1:40, :], AX.max)
            nc.vector.tensor_tensor(qx[:, 1:40, :], qx[:, 1:40, :],
                                    lamx[:, 1:40, :], AX.min)
            for e, (xs0, xs1) in ((0, (1, 0)), (40, (41, 40))):
                sl = slice(e, e + 1)
                nc.vector.tensor_sub(g[:, sl, :], xbar[:, xs0:xs0 + 1, 0:40],
                                     xbar[:, xs1:xs1 + 1, 0:40])
                nc.vector.tensor_add(g[:, sl, :], g[:, sl, :], qx[:, sl, :])
                nc.vector.tensor_tensor(qx[:, sl, :], g[:, sl, :],
                                        nlamx[:, sl, :], AX.max)
                nc.vector.tensor_tensor(qx[:, sl, :], qx[:, sl, :],
                                        lamx[:, sl, :], AX.min)

            # --- qy/qt on PE ---
            ps_y, ps_t = [], []
            for c in range(4):
                sl = slice(1 + NCH * c, 1 + NCH * (c + 1))
                slq = slice(NCH * c, NCH * (c + 1))
                ps = psum.tile([128, NCH, NYB], F32, tag="ps")
                nc.tensor.matmul(ps[:], W["w_i"][:], qy[:, slq, 1:41],
                                 start=True, stop=False)
                nc.tensor.matmul(ps[:], W["w_dy"][:], xbar[:, sl, 0:40],
                                 start=False, stop=False)
                nc.tensor.matmul(ps[:], W["w_cy"][:], xbar[:, sl, 1:41],
                                 start=False, stop=True)
                ps_y.append(ps)
            for c in range(4):
                sl = slice(1 + NCH * c, 1 + NCH * (c + 1))
                slq = slice(NCH * c, NCH * (c + 1))
                ps = psum.tile([128, NCH, NYB], F32, tag="ps")
                nc.tensor.matmul(ps[:], W["w_i"][:], qt[:, slq, :],
                                 start=True, stop=False)
                nc.tensor.matmul(ps[:], W["w_dt"][:], xbar[:, sl, 0:40],
                                 start=False, stop=True)
                ps_t.append(ps)
            for c in range(4):
                slq = slice(NCH * c, NCH * (c + 1))
                sl1 = slice(1 + NCH * c, 1 + NCH * (c + 1))
                nc.vector.tensor_tensor(qy[:, slq, 1:41], ps_y[c][:],
                                        nlamy[:, slq, :], AX.max)
            nc.vector.tensor_tensor(qy[:, :, 1:41], qy[:, :, 1:41],
                                    lamy[:], AX.min)
            nc.vector.tensor_copy(qy[:, :, 0:1], qy[:, :, 40:41])
            for c in range(4):
                slq = slice(NCH * c, NCH * (c + 1))
                nc.vector.tensor_tensor(qt[:, slq, :], ps_t[c][:],
                                        nlamt[:, slq, :], AX.max)
            nc.vector.tensor_tensor(qt[:], qt[:], lamt[:], AX.min)

            # --- p-phase ---
            t1 = spool.tile([128, NXS, NYB], F32, tag="t1")
            nc.vector.scalar_tensor_tensor(t1[:], xbar[:, 1:41, 0:40], a_,
                                           cxn[:], AX.mult, AX.subtract)
            nc.vector.scalar_tensor_tensor(mt[:], mt[:], a_, t1[:],
                                           AX.mult, AX.add)

            # --- D + x-phase, edge chunks (0, 3) last only in xbar emit ---
            v = spool.tile([128, NXS, NYB], F32, tag="v")
            x1 = xpool.tile([128, NXS, NYB], F32, tag="x")
            for c in (1, 2, 0, 3):
                sl = slice(1 + NCH * c, 1 + NCH * (c + 1))
                slq = slice(NCH * c, NCH * (c + 1))          # qx[x-1]
                slq1 = slice(NCH * c + 1, NCH * (c + 1) + 1)  # qx[x]
                ps = psum.tile([128, NCH, NYB], F32, tag="ps")
                nc.tensor.matmul(ps[:], W["w_dyh"][:], qy[:, slq, 1:41],
                                 start=True, stop=False)
                nc.tensor.matmul(ps[:], W["w_cyh"][:], qy[:, slq, 0:40],
                                 start=False, stop=False)
                nc.tensor.matmul(ps[:], W["w_dth"][:], qt[:, slq, :],
                                 start=False, stop=False)
                nc.tensor.matmul(ps[:], W["w_i"][:], qx[:, slq, :],
                                 start=False, stop=False)
                nc.tensor.matmul(ps[:], W["w_ni"][:], qx[:, slq1, :],
                                 start=False, stop=True)
                nc.vector.tensor_add(v[:, slq, :], mt[:, slq, :], ps[:])
                nc.vector.scalar_tensor_tensor(
                    x1[:, slq, :], v[:, slq, :], -c2, x0[:, slq, :],
                    AX.mult, AX.add)

            if k < T - 1:
                vb = spool.tile([128, NXS, NYB], F32, tag="vb")
                # edge columns of xbar' first, then launch the exchange
                for col in (0, 39):
                    slc = slice(col, col + 1)
                    slx = slice(col + 1, col + 2)
                    nc.vector.tensor_sub(vb[:, slc, :], x1[:, slc, :],
                                         x0[:, slc, :])
                    nc.vector.scalar_tensor_tensor(
                        xbar[:, slx, 0:40], vb[:, slc, :], th, x1[:, slc, :],
                        AX.mult, AX.add)
                bout = exchange(k + 1)
                nc.vector.tensor_sub(vb[:, 1:39, :], x1[:, 1:39, :],
                                     x0[:, 1:39, :])
                nc.vector.scalar_tensor_tensor(
                    xbar[:, 2:40, 0:40], vb[:, 1:39, :], th, x1[:, 1:39, :],
                    AX.mult, AX.add)
                nc.vector.tensor_copy(xbar[:, 1:41, 40:41],
                                      xbar[:, 1:41, 0:1])
            x0 = x1

        nc.sync.dma_start(out_dram[:], x0[:])

    nc.compile()
    return nc


@lru_cache(maxsize=4)
def _compiled(scalars, T):
    return _build_nc(scalars, T)


def _make_in_maps(x, lambda_map, scalars, sig):
    import ml_dtypes
    bf = ml_dtypes.bfloat16
    stats = _stationaries()
    a_, c2, th = scalars
    in_maps = []
    for rank in range(8):
        mbi, pos = rank // 4, rank % 4
        s = pos * NXS
        xs = slice(s, s + NXS)
        xn = np.ascontiguousarray(x[mbi, 0, xs]).astype(np.float32)
        lam = lambda_map[mbi].astype(np.float32) / np.float32(sig)
        # x-channel lambda on the 41-wide overlap slab [s-1, s+40)
        idx = [(s - 1 + j) % 160 for j in range(NXS + 1)]
        lx = lam[0][idx]
        nxt, prv = (pos + 1) % 4, (pos - 1) % 4
        mhi = np.zeros((128, 8), np.float32)
        mlo = np.zeros((128, 8), np.float32)
        mhi[:, 2 * nxt] = 1.0        # next's first plane -> halo_hi
        mlo[:, 2 * prv + 1] = 1.0    # prev's last plane  -> halo_lo
        m = dict(
            xb0=to_dev(xn).astype(bf),
            x00=to_dev(xn),
            mt0=to_dev(xn / np.float32(sig)),
            cxn=to_dev(np.float32(a_) * xn),
            lamx=to_dev(lx).astype(bf), nlamx=to_dev(-lx).astype(bf),
            lamy=to_dev(lam[1][xs]).astype(bf),
            nlamy=to_dev(-lam[1][xs]).astype(bf),
            lamt=to_dev(lam[2][xs]).astype(bf),
            nlamt=to_dev(-lam[2][xs]).astype(bf),
            mskhi=mhi, msklo=mlo,
        )
        m.update({k: v.astype(bf) for k, v in stats.items()})
        in_maps.append(m)
    return in_maps


def kernel(x, lambda_map, tau, sigma, theta):
    x = np.asarray(x, dtype=np.float32)
    lambda_map = np.asarray(lambda_map, dtype=np.float32)
    L = math.sqrt(13.0)
    sig = float(1.0 / (1.0 + math.exp(-float(np.asarray(sigma)[0])))) / L
    ta = float(1.0 / (1.0 + math.exp(-float(np.asarray(tau)[0])))) / L
    th = float(1.0 / (1.0 + math.exp(-float(np.asarray(theta)[0]))))
    a_ = 1.0 / (1.0 + sig)
    c2 = ta * sig
    scalars = tuple(float(np.float32(v)) for v in (a_, c2, th))

    nc = _compiled(scalars, T_ITERS)
    in_maps = _make_in_maps(x, lambda_map, scalars, sig)
    res = run_bass_kernel_spmd(nc, in_maps, core_ids=list(range(8)),
                               trace=TRACE)
    global _LAST_RESULTS
    _LAST_RESULTS = res

    out = np.zeros((2, 1, 160, 160, 32), np.float32)
    for rank in range(8):
        mbi, pos = rank // 4, rank % 4
        s = pos * NXS
        out[mbi, 0, s:s + NXS] = from_dev(res.results[rank]["out"])
    return out


# revision 19
# speedup vs baseline: 1.4119x; 1.0247x over previous
"""Trainium2 Bass kernel for nn_DynamicImagePrimalDualNN.

T=128 primal-dual iterations over (2,1,160,160,32) with circular FD stencils.

Distribution: mb(2) x x-slabs(4) = 8 cores (ranks 0-3 = image 0, 4-7 = image
1; slab = rank%4). y and t stay core-local.

One AllGather per iteration: the dual variable qx is kept on the overlapping
slab [s-1, s+40) (one column redundantly computed by both neighbours), which
makes grad_GH fully local; only xbar needs halos, and both its planes
(first/last real column) are exchanged together in a single 4-rank AllGather
launched at the end of the previous iteration. Edge-column work is scheduled
late so the collective hides under bulk compute. All cross-iteration
dependencies are ordinary Tile-tracked tensor accesses - no manual sems.

Per-core layout: partitions p = (y%4)*32 + t (all 128 used);
free = (x_slot, yb). y/t stencils run on the TensorEngine via exact +-1
stationaries (circular yb handled by one pad column, circular t inside the
stationary); x stencils are DVE free-dim shifts.

Rescaled state so every scalar is an fp32 stt immediate:
  mt = p/sig,  Q = q/sig,  x0 raw.
  mt' = a*mt + a*xbar - cxn          (a = 1/(1+sig), cxn = a*xnoisy)
  Q'  = clip(Q + grad(xbar), lam/sig)
  x1  = x0 - c2*(mt' + div(Q'))      (c2 = ta*sig)
  xbar'= x1 + th*(x1 - x0)
"""

import math
from contextlib import ExitStack
from functools import lru_cache

import numpy as np

import concourse.bass as bass
import concourse.tile as tile
from concourse import bacc, mybir
from concourse.bass_utils import run_bass_kernel_spmd

F32 = mybir.dt.float32
BF = mybir.dt.bfloat16
AX = mybir.AluOpType

T_ITERS = 128
TRACE = False
_LAST_RESULTS = None
NXS = 40          # x-slab width per core
NYB = 40          # y blocks (y = 4*yb + my)
NCH = 10          # x-chunk width for PSUM-bank-sized matmuls
GROUPS = [[0, 1, 2, 3], [4, 5, 6, 7]]


def _pidx(m, t):
    return m * 32 + t


def _stationaries():
    """(128,128) matrices W[p_in, p_out]; matmul computes out[i] = sum_k W[k,i] in[k]."""
    I = np.eye(128, dtype=np.float32)
    dy = -np.eye(128, dtype=np.float32)
    cy = np.zeros((128, 128), np.float32)
    dt = -np.eye(128, dtype=np.float32)
    dyh = -np.eye(128, dtype=np.float32)
    cyh = np.zeros((128, 128), np.float32)
    dth = -np.eye(128, dtype=np.float32)
    for t in range(32):
        for m in range(3):
            dy[_pidx(m + 1, t), _pidx(m, t)] += 1.0
        cy[_pidx(0, t), _pidx(3, t)] = 1.0
        for m in range(1, 4):
            dyh[_pidx(m - 1, t), _pidx(m, t)] += 1.0
        cyh[_pidx(3, t), _pidx(0, t)] = 1.0
        for m in range(4):
            dt[_pidx(m, (t + 1) % 32), _pidx(m, t)] += 1.0
            dth[_pidx(m, (t - 1) % 32), _pidx(m, t)] += 1.0
    return dict(w_i=I, w_ni=-I, w_dy=dy, w_cy=cy, w_dt=dt, w_dyh=dyh,
                w_cyh=cyh, w_dth=dth)


def to_dev(v):
    """(xs, 160y, 32t) -> (128, xs, 40yb) with p=(y%4)*32+t."""
    xs = v.shape[0]
    return np.ascontiguousarray(
        v.reshape(xs, NYB, 4, 32).transpose(2, 3, 0, 1).reshape(128, xs, NYB))


def from_dev(v):
    """(128, xs, 40yb) -> (xs, 160y, 32t)."""
    xs = v.shape[1]
    return np.ascontiguousarray(
        v.reshape(4, 32, xs, NYB).transpose(2, 3, 0, 1).reshape(xs, 160, 32))


def _build_nc(scalars, T=T_ITERS):
    a_, c2, th = scalars
    nc = bacc.Bacc("TRN2", target_bir_lowering=False, debug=False,
                   num_devices=8)

    dp = {}
    dp["xb0"] = nc.dram_tensor("xb0", [128, NXS, NYB], BF,
                               kind="ExternalInput")
    for name in ("x00", "z00", "mt0", "cxn"):
        dp[name] = nc.dram_tensor(name, [128, NXS, NYB], F32,
                                  kind="ExternalInput")
    # x-channel lambda covers the 41-wide overlap slab
    for name in ("lamx", "nlamx"):
        dp[name] = nc.dram_tensor(name, [128, NXS + 1, NYB], BF,
                                  kind="ExternalInput")
    for name in ("lamy", "nlamy", "lamt", "nlamt"):
        dp[name] = nc.dram_tensor(name, [128, NXS, NYB], BF,
                                  kind="ExternalInput")
    # (128, 8) one-hot masks over gathered slots (slot = rank_in_group*2 + e)
    for name in ("mskhi", "msklo"):
        dp[name] = nc.dram_tensor(name, [128, 8], F32, kind="ExternalInput")
    wnames = list(_stationaries().keys())
    for name in wnames:
        dp[name] = nc.dram_tensor(name, [128, 128], BF, kind="ExternalInput")
    for name in ("w_i32", "w_nx32"):
        dp[name] = nc.dram_tensor(name, [128, 128], F32,
                                  kind="ExternalInput")
    out_dram = nc.dram_tensor("out", [128, NXS, NYB], F32,
                              kind="ExternalOutput")

    with tile.TileContext(nc) as tc, ExitStack() as es:
        state = es.enter_context(tc.tile_pool(name="state", bufs=1))
        xpool = es.enter_context(tc.tile_pool(name="xp", bufs=2))
        spool = es.enter_context(tc.tile_pool(name="scratch", bufs=2))
        dpool = es.enter_context(tc.tile_pool(name="dram", bufs=2,
                                              space="DRAM"))
        gpool = es.enter_context(tc.tile_pool(name="gath", bufs=2))
        psum = es.enter_context(
            tc.tile_pool(name="psum", bufs=8, space=bass.MemorySpace.PSUM))

        # xbar: x slots 0=halo_lo, 1..40 real, 41=halo_hi; yb col 40 =
        # pad(yb0), col 41 unused (even stride keeps bf16 2x alignment)
        xbar = state.tile([128, NXS + 2, NYB + 2], BF, tag="xbar")
        # qx on the 41-wide overlap slab (col j = global x s-1+j), no halos
        qx = state.tile([128, NXS + 1, NYB], BF, tag="qx")
        # qy: yb col 0 = pad(yb39), real yb at cols 1..40, col 41 unused
        qy = state.tile([128, NXS, NYB + 2], BF, tag="qy")
        qt = state.tile([128, NXS, NYB], BF, tag="qt")
        mt = state.tile([128, NXS, NYB], F32, tag="mt")
        cxn = state.tile([128, NXS, NYB], F32, tag="cxn")
        lamx = state.tile([128, NXS + 1, NYB], BF, tag="lamx")
        nlamx = state.tile([128, NXS + 1, NYB], BF, tag="nlamx")
        lamy = state.tile([128, NXS, NYB], BF, tag="lamy")
        nlamy = state.tile([128, NXS, NYB], BF, tag="nlamy")
        lamt = state.tile([128, NXS, NYB], BF, tag="lamt")
        nlamt = state.tile([128, NXS, NYB], BF, tag="nlamt")
        mskhi = state.tile([128, 8], F32, tag="mskhi")
        msklo = state.tile([128, 8], F32, tag="msklo")
        W = {n: state.tile([128, 128], BF, tag=n, name=f"w_{n}")
             for n in wnames}
        W32 = {n: state.tile([128, 128], F32, tag=n, name=f"w32_{n}")
               for n in ("w_i32", "w_nx32")}

        nc.sync.dma_start(xbar[:, 1:41, 0:40], dp["xb0"][:])
        x0 = xpool.tile([128, NXS, NYB], F32, tag="x")
        nc.sync.dma_start(x0[:], dp["x00"][:])
        zt = xpool.tile([128, NXS, NYB], F32, tag="z")
        nc.sync.dma_start(zt[:], dp["z00"][:])
        nc.sync.dma_start(mt[:], dp["mt0"][:])
        nc.sync.dma_start(cxn[:], dp["cxn"][:])
        for nm, tl in (("lamx", lamx), ("nlamx", nlamx), ("lamy", lamy),
                       ("nlamy", nlamy), ("lamt", lamt), ("nlamt", nlamt),
                       ("mskhi", mskhi), ("msklo", msklo)):
            nc.sync.dma_start(tl[:], dp[nm][:])
        for n in wnames:
            nc.sync.dma_start(W[n][:], dp[n][:])
        for n in ("w_i32", "w_nx32"):
            nc.sync.dma_start(W32[n][:], dp[n][:])
        nc.vector.memset(qx[:], 0.0)
        nc.vector.memset(qy[:], 0.0)
        nc.vector.memset(qt[:], 0.0)
        nc.vector.tensor_copy(xbar[:, 1:41, 40:41], xbar[:, 1:41, 0:1])

        def exchange(round_idx):
            """AG of my (first,last) xbar planes; returns gathered sbuf tile."""
            bin_ = dpool.tile([2, 128, NYB], BF, tag="bin",
                              name=f"bin{round_idx}")
            bout = dpool.tile([8, 128, NYB], BF, tag="bout",
                              name=f"bout{round_idx}")
            nc.sync.dma_start(bin_[0], xbar[:, 1, 0:40])
            nc.sync.dma_start(bin_[1], xbar[:, 40, 0:40])
            nc.gpsimd.collective_compute(
                "AllGather", AX.bypass, replica_groups=GROUPS,
                ins=[bin_[:]], outs=[bout[:]])
            return bout

        def recv(bout):
            """DMA gathered planes to SBUF and mask-combine into xbar halos."""
            gath = gpool.tile([128, 8, NYB], BF, tag="gath")
            nc.sync.dma_start(gath[:], bout[:].transpose([1, 0, 2]))
            hi = xbar[:, 41, 0:40]
            lo = xbar[:, 0, 0:40]
            nc.vector.tensor_scalar(hi, gath[:, 0, :], mskhi[:, 0:1],
                                    None, AX.mult)
            nc.vector.tensor_scalar(lo, gath[:, 1, :], msklo[:, 1:2],
                                    None, AX.mult)
            for j in (1, 2, 3):
                nc.vector.scalar_tensor_tensor(
                    hi, gath[:, 2 * j, :], mskhi[:, 2 * j:2 * j + 1],
                    hi, AX.mult, AX.add)
                nc.vector.scalar_tensor_tensor(
                    lo, gath[:, 2 * j + 1, :],
                    msklo[:, 2 * j + 1:2 * j + 2], lo, AX.mult, AX.add)

        bout = exchange(0)

        for k in range(T):
            recv(bout)

            # --- qx chain: interior (cols 1..39) free of halos; edges last ---
            g = spool.tile([128, NXS + 1, NYB], BF, tag="g")
            nc.vector.tensor_sub(g[:, 1:40, :], xbar[:, 2:41, 0:40],
                                 xbar[:, 1:40, 0:40])
            nc.vector.tensor_add(g[:, 1:40, :], g[:, 1:40, :], qx[:, 1:40, :])
            nc.vector.tensor_tensor(qx[:, 1:40, :], g[:, 1:40, :],
                                    nlamx[:, 1:40, :], AX.max)
            nc.vector.tensor_tensor(qx[:, 1:40, :], qx[:, 1:40, :],
                                    lamx[:, 1:40, :], AX.min)
            sle = slice(0, 41, 40)
            nc.vector.tensor_sub(g[:, sle, :], xbar[:, 1:42:40, 0:40],
                                 xbar[:, 0:41:40, 0:40])
            nc.vector.tensor_add(g[:, sle, :], g[:, sle, :], qx[:, sle, :])
            nc.vector.tensor_tensor(qx[:, sle, :], g[:, sle, :],
                                    nlamx[:, sle, :], AX.max)
            nc.vector.tensor_tensor(qx[:, sle, :], qx[:, sle, :],
                                    lamx[:, sle, :], AX.min)

            # --- qy/qt on PE ---
            ps_y, ps_t = [], []
            for c in range(4):
                sl = slice(1 + NCH * c, 1 + NCH * (c + 1))
                slq = slice(NCH * c, NCH * (c + 1))
                ps = psum.tile([128, NCH, NYB], F32, tag="ps")
                nc.tensor.matmul(ps[:], W["w_i"][:], qy[:, slq, 1:41],
                                 start=True, stop=False)
                nc.tensor.matmul(ps[:], W["w_dy"][:], xbar[:, sl, 0:40],
                                 start=False, stop=False)
                nc.tensor.matmul(ps[:], W["w_cy"][:], xbar[:, sl, 1:41],
                                 start=False, stop=True)
                ps_y.append(ps)
            for c in range(4):
                sl = slice(1 + NCH * c, 1 + NCH * (c + 1))
                slq = slice(NCH * c, NCH * (c + 1))
                ps = psum.tile([128, NCH, NYB], F32, tag="ps")
                nc.tensor.matmul(ps[:], W["w_i"][:], qt[:, slq, :],
                                 start=True, stop=False)
                nc.tensor.matmul(ps[:], W["w_dt"][:], xbar[:, sl, 0:40],
                                 start=False, stop=True)
                ps_t.append(ps)
            qsy = spool.tile([128, NXS, NYB], BF, tag="qsy")
            qst = spool.tile([128, NXS, NYB], BF, tag="qst")
            for c in range(4):
                slq = slice(NCH * c, NCH * (c + 1))
                nc.scalar.activation(qsy[:, slq, :], ps_y[c][:],
                                     mybir.ActivationFunctionType.Copy)
                nc.scalar.activation(qst[:, slq, :], ps_t[c][:],
                                     mybir.ActivationFunctionType.Copy)
            nc.vector.tensor_tensor(qy[:, :, 1:41], qsy[:], nlamy[:], AX.max)
            nc.vector.tensor_tensor(qy[:, :, 1:41], qy[:, :, 1:41],
                                    lamy[:], AX.min)
            nc.scalar.copy(qy[:, :, 0:1], qy[:, :, 40:41])
            nc.vector.tensor_tensor(qt[:], qst[:], nlamt[:], AX.max)
            nc.vector.tensor_tensor(qt[:], qt[:], lamt[:], AX.min)

            # --- p-phase ---
            t1 = spool.tile([128, NXS, NYB], F32, tag="t1")
            nc.vector.scalar_tensor_tensor(t1[:], xbar[:, 1:41, 0:40], a_,
                                           cxn[:], AX.mult, AX.subtract)
            nc.vector.scalar_tensor_tensor(mt[:], mt[:], a_, t1[:],
                                           AX.mult, AX.add)

            # --- D + x-phase: PE accumulates mt + D - (1/c2)x0; ACT scales
            # out x1 = -c2*ps and z~' = th*x1; DVE only emits xbar' ---
            x1 = xpool.tile([128, NXS, NYB], F32, tag="x")
            zn = xpool.tile([128, NXS, NYB], F32, tag="z")
            for c in (1, 2, 0, 3):
                slq = slice(NCH * c, NCH * (c + 1))          # qx[x-1]
                slq1 = slice(NCH * c + 1, NCH * (c + 1) + 1)  # qx[x]
                ps = psum.tile([128, NCH, NYB], F32, tag="ps")
                nc.tensor.matmul(ps[:], W["w_dyh"][:], qy[:, slq, 1:41],
                                 start=True, stop=False)
                nc.tensor.matmul(ps[:], W["w_cyh"][:], qy[:, slq, 0:40],
                                 start=False, stop=False)
                nc.tensor.matmul(ps[:], W["w_dth"][:], qt[:, slq, :],
                                 start=False, stop=False)
                nc.tensor.matmul(ps[:], W["w_i"][:], qx[:, slq, :],
                                 start=False, stop=False)
                nc.tensor.matmul(ps[:], W["w_ni"][:], qx[:, slq1, :],
                                 start=False, stop=False)
                nc.tensor.matmul(ps[:], W32["w_i32"][:], mt[:, slq, :],
                                 start=False, stop=False)
                nc.tensor.matmul(ps[:], W32["w_nx32"][:], x0[:, slq, :],
                                 start=False, stop=True)
                nc.scalar.activation(x1[:, slq, :], ps[:],
                                     mybir.ActivationFunctionType.Copy,
                                     scale=-c2)
                nc.scalar.activation(zn[:, slq, :], ps[:],
                                     mybir.ActivationFunctionType.Copy,
                                     scale=-c2 * th)

            if k < T - 1:
                # edge columns of xbar' first (strided 2-col op), then AG
                nc.vector.scalar_tensor_tensor(
                    xbar[:, 1:41:39, 0:40], x1[:, 0:40:39, :], 1.0 + th,
                    zt[:, 0:40:39, :], AX.mult, AX.subtract)
                bout = exchange(k + 1)
                nc.vector.scalar_tensor_tensor(
                    xbar[:, 2:40, 0:40], x1[:, 1:39, :], 1.0 + th,
                    zt[:, 1:39, :], AX.mult, AX.subtract)
                nc.scalar.copy(xbar[:, 1:41, 40:41], xbar[:, 1:41, 0:1])
            x0 = x1
            zt = zn

        nc.sync.dma_start(out_dram[:], x0[:])

    nc.compile()
    return nc


@lru_cache(maxsize=4)
def _compiled(scalars, T):
    return _build_nc(scalars, T)


def _make_in_maps(x, lambda_map, scalars, sig):
    import ml_dtypes
    bf = ml_dtypes.bfloat16
    stats = _stationaries()
    a_, c2, th = scalars
    in_maps = []
    for rank in range(8):
        mbi, pos = rank // 4, rank % 4
        s = pos * NXS
        xs = slice(s, s + NXS)
        xn = np.ascontiguousarray(x[mbi, 0, xs]).astype(np.float32)
        lam = lambda_map[mbi].astype(np.float32) / np.float32(sig)
        # x-channel lambda on the 41-wide overlap slab [s-1, s+40)
        idx = [(s - 1 + j) % 160 for j in range(NXS + 1)]
        lx = lam[0][idx]
        nxt, prv = (pos + 1) % 4, (pos - 1) % 4
        mhi = np.zeros((128, 8), np.float32)
        mlo = np.zeros((128, 8), np.float32)
        mhi[:, 2 * nxt] = 1.0        # next's first plane -> halo_hi
        mlo[:, 2 * prv + 1] = 1.0    # prev's last plane  -> halo_lo
        m = dict(
            xb0=to_dev(xn).astype(bf),
            x00=to_dev(xn),
            z00=to_dev(np.float32(th) * xn),
            mt0=to_dev(xn / np.float32(sig)),
            cxn=to_dev(np.float32(a_) * xn),
            lamx=to_dev(lx).astype(bf), nlamx=to_dev(-lx).astype(bf),
            lamy=to_dev(lam[1][xs]).astype(bf),
            nlamy=to_dev(-lam[1][xs]).astype(bf),
            lamt=to_dev(lam[2][xs]).astype(bf),
            nlamt=to_dev(-lam[2][xs]).astype(bf),
            mskhi=mhi, msklo=mlo,
        )
        m.update({k: v.astype(bf) for k, v in stats.items()})
        m["w_i32"] = np.eye(128, dtype=np.float32)
        m["w_nx32"] = (-1.0 / np.float32(c2)) * np.eye(128, dtype=np.float32)
        in_maps.append(m)
    return in_maps


def kernel(x, lambda_map, tau, sigma, theta):
    x = np.asarray(x, dtype=np.float32)
    lambda_map = np.asarray(lambda_map, dtype=np.float32)
    L = math.sqrt(13.0)
    sig = float(1.0 / (1.0 + math.exp(-float(np.asarray(sigma)[0])))) / L
    ta = float(1.0 / (1.0 + math.exp(-float(np.asarray(tau)[0])))) / L
    th = float(1.0 / (1.0 + math.exp(-float(np.asarray(theta)[0]))))
    a_ = 1.0 / (1.0 + sig)
    c2 = ta * sig
    scalars = tuple(float(np.float32(v)) for v in (a_, c2, th))

    nc = _compiled(scalars, T_ITERS)
    in_maps = _make_in_maps(x, lambda_map, scalars, sig)
    res = run_bass_kernel_spmd(nc, in_maps, core_ids=list(range(8)),
                               trace=TRACE)
    global _LAST_RESULTS
    _LAST_RESULTS = res

    out = np.zeros((2, 1, 160, 160, 32), np.float32)
    for rank in range(8):
        mbi, pos = rank // 4, rank % 4
        s = pos * NXS
        out[mbi, 0, s:s + NXS] = from_dev(res.results[rank]["out"])
    return out
